# revision 11
# baseline (speedup 1.0000x reference)
"""Trainium2 Bass kernel for nn_AttnConvolutionalDecoder.

Data-parallel over batch: B=16 -> 2 batch elements per core on 8 NeuronCores.
All activations are kept channel-major (channels on SBUF partitions, time on
the free axis), which makes the causal conv 3 accumulating matmuls over
shifted slices of the same SBUF buffer (no transposes anywhere).

Attention uses the algebraic identity
    ctx = (d @ G_b) / (d . m_b),   G_b = enc_b^T enc_b,  m_b = sum_s enc_b[s]
(valid because the "attention" normalizes raw scores by their plain sum), and
enc2in is folded into G: ctx_proj = (d @ (G_b @ enc2in^T)) / (d . m_b).
The folded matrices Gfold[i,b] are computed once at startup and staged in
DRAM.

Matmuls run in float32r (fp32 operands truncated to ~FP22 inside the PE) which
streams at full bf16-rate for N>=256.
"""

import numpy as np

L, KW, C, D, E = 4, 3, 512, 512, 512
T, B, S, V, MAXT = 1024, 16, 512, 32, 1024
NCORES = 8
BPC = B // NCORES          # batch elements per core
NC_T, NCH = 2, 4           # time chunks of 512; channel tiles of 128
P = 128
TC = T // NC_T             # 512

_compiled = None


def _build_nc(reps=1):
    import concourse.bacc as bacc
    import concourse.mybir as mybir
    import concourse.tile as tile

    F32 = mybir.dt.float32
    F32R = mybir.dt.float32r
    AF = mybir.ActivationFunctionType
    OP = mybir.AluOpType

    nc = bacc.Bacc("TRN2", target_bir_lowering=False, debug=False,
                   num_devices=NCORES)

    dt = nc.dram_tensor
    # conv / linear weights, pre-transposed + tiled on host:
    #   lhsT block layout [.., kc, m, 128(k-part), 128(m-free)]
    Wglu = dt("Wglu", [L, KW, NCH, NCH, P, P], F32R, kind="ExternalInput").ap()
    Wid = dt("Wid", [L, KW, NCH, NCH, P, P], F32R, kind="ExternalInput").ap()
    Wres = dt("Wres", [L, NCH, NCH, P, P], F32R, kind="ExternalInput").ap()
    Winres = dt("Winres", [L, NCH, NCH, P, P], F32R, kind="ExternalInput").ap()
    Win2enc = dt("Win2enc", [L, NCH, NCH, P, P], F32R, kind="ExternalInput").ap()
    Wlab2enc = dt("Wlab2enc", [L, NCH, NCH, P, P], F32R, kind="ExternalInput").ap()
    Wenc2in_r = dt("Wenc2in_r", [L, NCH, P, C], F32R, kind="ExternalInput").ap()
    enc_lhs = dt("enc_lhs", [BPC, NCH, NCH, P, P], F32R, kind="ExternalInput").ap()
    enc_rhs = dt("enc_rhs", [BPC, NCH, P, E], F32R, kind="ExternalInput").ap()
    onehot = dt("onehot", [BPC, V, T], F32R, kind="ExternalInput").ap()
    labelW = dt("labelW", [V, D], F32R, kind="ExternalInput").ap()
    timeT = dt("timeT", [NCH, P, T], F32R, kind="ExternalInput").ap()
    Wout = dt("Wout", [NCH, P, V], F32R, kind="ExternalInput").ap()
    Woutres = dt("Woutres", [NCH, P, V], F32R, kind="ExternalInput").ap()
    # bias vectors (f32): [L, NCH, P, 1]
    bglu = dt("bglu", [L, NCH, P, 1], F32, kind="ExternalInput").ap()
    bid = dt("bid", [L, NCH, P, 1], F32, kind="ExternalInput").ap()
    bres = dt("bres", [L, NCH, P, 1], F32, kind="ExternalInput").ap()
    bbeta = dt("bbeta", [L, NCH, P, 1], F32, kind="ExternalInput").ap()
    b6 = dt("b6", [L, NCH, P, 1], F32, kind="ExternalInput").ap()
    bout = dt("bout", [V, 1], F32, kind="ExternalInput").ap()
    onesv = dt("onesv", [P, 2], F32R, kind="ExternalInput").ap()
    zerov = dt("zerov", [P, 2], F32R, kind="ExternalInput").ap()

    out = dt("out", [BPC, V, T], F32, kind="ExternalOutput").ap()

    with tile.TileContext(nc) as tc:
        from contextlib import ExitStack
        es = ExitStack()

        def pool(name, bufs, space="SBUF"):
            return es.enter_context(
                tc.tile_pool(name=name, bufs=bufs, space=space))

        pers = pool("pers", 1)          # persistent tiles (unique tags)
        dram = pool("dram", 1, space="DRAM")
        wA = pool("wA", 36)             # [128,128] lhsT weight tiles
        wC = pool("wC", 18)             # [128,512] rhs weight / emb tiles
        tmp = pool("tmp", 4)            # DVE scratch
        ps = pool("ps", 8, space="PSUM")

        def mm(out_ap, lhsT, rhs, start, stop):
            nc.tensor.matmul(out_ap, lhsT, rhs, start=start, stop=stop)

        # ---- persistent tiles ----
        h = [[pers.tile([P, T + 2], F32R, tag=f"h_{b}_{m}", name=f"h_{b}_{m}")
              for m in range(NCH)] for b in range(BPC)]
        mrep = [[pers.tile([P, P], F32R, tag=f"mr_{b}_{m}", name=f"mr_{b}_{m}")
                 for m in range(NCH)] for b in range(BPC)]
        bias_t = {}
        for nm, src in (("bglu", bglu), ("bid", bid), ("bres", bres),
                        ("bbeta", bbeta), ("b6", b6)):
            for i in range(L):
                for m in range(NCH):
                    bt = pers.tile([P, 1], F32, tag=f"{nm}_{i}_{m}",
                                   name=f"{nm}_{i}_{m}")
                    nc.sync.dma_start(out=bt, in_=src[i, m])
                    bias_t[(nm, i, m)] = bt
        bout_t = pers.tile([V, 1], F32, tag="bout", name="bout")
        nc.sync.dma_start(out=bout_t, in_=bout)
        ones_t = pers.tile([P, 2], F32R, tag="ones", name="ones")
        nc.sync.dma_start(out=ones_t, in_=onesv)
        for b in range(BPC):
            for m in range(NCH):
                nc.sync.dma_start(out=h[b][m][:, 0:2], in_=zerov)

        # DRAM staging: emb (channel-major) and folded attention matrices
        embd = [[dram.tile([P, T], F32R, tag=f"embd_{b}_{k}",
                           name=f"embd_{b}_{k}")
                 for k in range(NCH)] for b in range(BPC)]
        gfd = [[[dram.tile([P, C], F32R, tag=f"gfd_{i}_{b}_{m}",
                           name=f"gfd_{i}_{b}_{m}")
                 for m in range(NCH)] for b in range(BPC)] for i in range(L)]

        # ---- startup (scoped pool): emb, G, Gfold, mrep ----
        with tc.tile_pool(name="su", bufs=1) as su:
            lw_t = [su.tile([V, P], F32R, tag=f"lw_{m}", name=f"lw_{m}")
                    for m in range(NCH)]
            for m in range(NCH):
                nc.sync.dma_start(out=lw_t[m], in_=labelW[:, m * P:(m + 1) * P])
            oh_t = [su.tile([V, T], F32R, tag=f"oh_{b}", name=f"oh_{b}")
                    for b in range(BPC)]
            for b in range(BPC):
                nc.sync.dma_start(out=oh_t[b], in_=onehot[b])
            G = [[su.tile([P, E], F32R, tag=f"G_{b}_{m}", name=f"G_{b}_{m}")
                  for m in range(NCH)] for b in range(BPC)]

            for b in range(BPC):
                # emb = labelW one-hot matmul + timeT;  h <- emb; embd <- emb
                for kd in range(NCH):
                    for ch in range(NC_T):
                        tt = wC.tile([P, TC], F32R, tag="wC", name="wC")
                        nc.sync.dma_start(
                            out=tt, in_=timeT[kd, :, ch * TC:(ch + 1) * TC])
                        pe = ps.tile([P, TC], F32, tag="ps", name="ps")
                        mm(pe, lw_t[kd], oh_t[b][:, ch * TC:(ch + 1) * TC],
                           True, True)
                        et = tmp.tile([P, TC], F32R, tag="tmp", name="tmp")
                        nc.vector.tensor_tensor(et, pe, tt, OP.add)
                        nc.scalar.copy(
                            out=h[b][kd][:, 2 + ch * TC:2 + (ch + 1) * TC],
                            in_=et)
                        nc.sync.dma_start(
                            out=embd[b][kd][:, ch * TC:(ch + 1) * TC], in_=et)
                # Gram matrix G_b = enc_b^T enc_b  (E x E), m_b replicated
                el = [[None] * NCH for _ in range(NCH)]
                for sc in range(NCH):
                    for m in range(NCH):
                        t = wA.tile([P, P], F32R, tag="wA", name="wA")
                        nc.sync.dma_start(out=t, in_=enc_lhs[b, sc, m])
                        el[sc][m] = t
                er = []
                for sc in range(NCH):
                    t = wC.tile([P, E], F32R, tag="wC", name="wC")
                    nc.sync.dma_start(out=t, in_=enc_rhs[b, sc])
                    er.append(t)
                for m in range(NCH):
                    pg = ps.tile([P, E], F32, tag="ps", name="ps")
                    for sc in range(NCH):
                        mm(pg, el[sc][m], er[sc], sc == 0, sc == NCH - 1)
                    nc.scalar.copy(out=G[b][m], in_=pg)
                for m in range(NCH):
                    pm = ps.tile([P, 2], F32, tag="ps", name="ps")
                    for sc in range(NCH):
                        mm(pm, el[sc][m], ones_t, sc == 0, sc == NCH - 1)
                    nc.vector.tensor_copy(out=mrep[b][m],
                                          in_=pm[:, 0:1].to_broadcast([P, P]))
            # Gfold[i,b] = G_b @ enc2in_w[i]^T  -> DRAM
            for i in range(L):
                e2r = []
                for kc in range(NCH):
                    t = wC.tile([P, C], F32R, tag="wC", name="wC")
                    nc.sync.dma_start(out=t, in_=Wenc2in_r[i, kc])
                    e2r.append(t)
                for b in range(BPC):
                    for m in range(NCH):
                        pf = ps.tile([P, C], F32, tag="ps", name="ps")
                        for kc in range(NCH):
                            # G symmetric: G[kc-block][:, m-block] is the lhsT
                            # block with e' on partitions
                            mm(pf, G[b][kc][:, m * P:(m + 1) * P], e2r[kc],
                               kc == 0, kc == NCH - 1)
                        gt = tmp.tile([P, C], F32R, tag="tmp", name="tmp")
                        nc.scalar.copy(out=gt, in_=pf)
                        nc.sync.dma_start(out=gfd[i][b][m], in_=gt)

        # ---- steady-state pools (reuse the startup pool's space) ----
        cvo = pool("cvo", 16)           # conv_out tiles [128,512] f32r
        dts = pool("dts", 16)           # d tiles [128,512] f32r
        gfp = pool("gfp", 10)           # Gfold tiles [128,512] f32r
        sgp = pool("sgp", 2)            # sigmoid(G) tiles f32
        rbp = pool("rbp", 4)            # 1/den broadcast tiles f32
        otp = pool("otp", 2)            # output staging [32,512] f32

        # ---- layers (reps>1 is a timing harness: restart from h=emb) ----
        for rep in range(reps):
            if rep > 0:
                for b in range(BPC):
                    for kd in range(NCH):
                        nc.sync.dma_start(out=h[b][kd][:, 2:2 + T],
                                          in_=embd[b][kd])
            for i in range(L):
                # stage A: conv_out = (X+bglu)*sigmoid(Gc+bid) + (R+bres)
                cv = [[[None] * NC_T for _ in range(NCH)] for _ in range(BPC)]
                for m in range(NCH):
                    wg = [[None] * NCH for _ in range(KW)]
                    wi = [[None] * NCH for _ in range(KW)]
                    wr = [None] * NCH
                    for tap in range(KW):
                        for kc in range(NCH):
                            t = wA.tile([P, P], F32R, tag="wA", name="wA")
                            nc.sync.dma_start(out=t, in_=Wglu[i, tap, kc, m])
                            wg[tap][kc] = t
                            t = wA.tile([P, P], F32R, tag="wA", name="wA")
                            nc.sync.dma_start(out=t, in_=Wid[i, tap, kc, m])
                            wi[tap][kc] = t
                    for kc in range(NCH):
                        t = wA.tile([P, P], F32R, tag="wA", name="wA")
                        nc.sync.dma_start(out=t, in_=Wres[i, kc, m])
                        wr[kc] = t
                    for b in range(BPC):
                        for ch in range(NC_T):
                            t0 = ch * TC
                            px = ps.tile([P, TC], F32, tag="ps", name="ps")
                            pg = ps.tile([P, TC], F32, tag="ps", name="ps")
                            pr = ps.tile([P, TC], F32, tag="ps", name="ps")
                            for wmat, pdst in ((wg, px), (wi, pg)):
                                n = 0
                                for tap in range(KW):
                                    for kc in range(NCH):
                                        mm(pdst[:, :], wmat[tap][kc],
                                           h[b][kc][:, t0 + tap:t0 + tap + TC],
                                           n == 0, n == KW * NCH - 1)
                                        n += 1
                            for kc in range(NCH):
                                mm(pr, wr[kc], h[b][kc][:, 2 + t0:2 + t0 + TC],
                                   kc == 0, kc == NCH - 1)
                            sg = sgp.tile([P, TC], F32, tag="sgp", name="sgp")
                            nc.scalar.activation(out=sg, in_=pg,
                                                 func=AF.Sigmoid,
                                                 bias=bias_t[("bid", i, m)],
                                                 scale=1.0)
                            t1 = tmp.tile([P, TC], F32, tag="tmp", name="tmp")
                            nc.vector.scalar_tensor_tensor(
                                out=t1, in0=px, scalar=bias_t[("bglu", i, m)],
                                in1=sg, op0=OP.add, op1=OP.mult)
                            cvt = cvo.tile([P, TC], F32R, tag="cvo", name="cvo")
                            nc.vector.scalar_tensor_tensor(
                                out=cvt, in0=pr, scalar=bias_t[("bres", i, m)],
                                in1=t1, op0=OP.add, op1=OP.add)
                            cv[b][m][ch] = cvt

                # stage B: d = conv_out@in2enc^T + emb@lab2enc^T + beta
                dti = [[[None] * NC_T for _ in range(NCH)] for _ in range(BPC)]
                emb_t = [[[None] * NC_T for _ in range(NCH)]
                         for _ in range(BPC)]
                for b in range(BPC):
                    for kd in range(NCH):
                        for ch in range(NC_T):
                            t = wC.tile([P, TC], F32R, tag="wC", name="wC")
                            nc.sync.dma_start(
                                out=t,
                                in_=embd[b][kd][:, ch * TC:(ch + 1) * TC])
                            emb_t[b][kd][ch] = t
                for m in range(NCH):
                    w2e = [None] * NCH
                    wl2 = [None] * NCH
                    for kc in range(NCH):
                        t = wA.tile([P, P], F32R, tag="wA", name="wA")
                        nc.sync.dma_start(out=t, in_=Win2enc[i, kc, m])
                        w2e[kc] = t
                        t = wA.tile([P, P], F32R, tag="wA", name="wA")
                        nc.sync.dma_start(out=t, in_=Wlab2enc[i, kc, m])
                        wl2[kc] = t
                    for b in range(BPC):
                        for ch in range(NC_T):
                            pd = ps.tile([P, TC], F32, tag="ps", name="ps")
                            for kc in range(NCH):
                                mm(pd, w2e[kc], cv[b][kc][ch], kc == 0, False)
                            for kd in range(NCH):
                                mm(pd, wl2[kd], emb_t[b][kd][ch],
                                   False, kd == NCH - 1)
                            dd = dts.tile([P, TC], F32R, tag="dts", name="dts")
                            nc.scalar.activation(
                                out=dd, in_=pd, func=AF.Identity,
                                bias=bias_t[("bbeta", i, m)], scale=1.0)
                            dti[b][m][ch] = dd

                # stage D: h = conv_out + (d@Gfold)/den + emb@inres^T + b6
                rb = [[None] * NC_T for _ in range(BPC)]
                gf = [[None] * NCH for _ in range(BPC)]
                for b in range(BPC):
                    for kc in range(NCH):
                        t = gfp.tile([P, C], F32R, tag="gfp", name="gfp")
                        nc.sync.dma_start(out=t, in_=gfd[i][b][kc])
                        gf[b][kc] = t
                    for ch in range(NC_T):
                        pden = ps.tile([P, TC], F32, tag="ps", name="ps")
                        for kc in range(NCH):
                            mm(pden, mrep[b][kc], dti[b][kc][ch],
                               kc == 0, kc == NCH - 1)
                        rt = rbp.tile([P, TC], F32, tag="rbp", name="rbp")
                        nc.vector.reciprocal(out=rt, in_=pden)
                        rb[b][ch] = rt
                for m in range(NCH):
                    wir = [None] * NCH
                    for kd in range(NCH):
                        t = wA.tile([P, P], F32R, tag="wA", name="wA")
                        nc.sync.dma_start(out=t, in_=Winres[i, kd, m])
                        wir[kd] = t
                    for b in range(BPC):
                        for ch in range(NC_T):
                            pc = ps.tile([P, TC], F32, tag="ps", name="ps")
                            for kc in range(NCH):
                                mm(pc, gf[b][kc][:, m * P:(m + 1) * P],
                                   dti[b][kc][ch], kc == 0, kc == NCH - 1)
                            p6 = ps.tile([P, TC], F32, tag="ps", name="ps")
                            for kd in range(NCH):
                                mm(p6, wir[kd], emb_t[b][kd][ch],
                                   kd == 0, kd == NCH - 1)
                            t1 = tmp.tile([P, TC], F32, tag="tmp", name="tmp")
                            nc.vector.tensor_tensor(t1, pc, rb[b][ch], OP.mult)
                            t2 = tmp.tile([P, TC], F32, tag="tmp", name="tmp")
                            nc.vector.tensor_tensor(t2, t1, cv[b][m][ch],
                                                    OP.add)
                            nc.vector.scalar_tensor_tensor(
                                out=h[b][m][:, 2 + ch * TC:2 + (ch + 1) * TC],
                                in0=p6, scalar=bias_t[("b6", i, m)],
                                in1=t2, op0=OP.add, op1=OP.add)

            # ---- output: out = h@out_proj^T + emb@out_res^T + bout ----
            wo = []
            wor = []
            for kc in range(NCH):
                t = wA.tile([P, V], F32R, tag="wAo", name="wAo")
                nc.sync.dma_start(out=t, in_=Wout[kc])
                wo.append(t)
                t = wA.tile([P, V], F32R, tag="wAo", name="wAo")
                nc.sync.dma_start(out=t, in_=Woutres[kc])
                wor.append(t)
            for b in range(BPC):
                for ch in range(NC_T):
                    em = []
                    for kd in range(NCH):
                        t = wC.tile([P, TC], F32R, tag="wC", name="wC")
                        nc.sync.dma_start(
                            out=t, in_=embd[b][kd][:, ch * TC:(ch + 1) * TC])
                        em.append(t)
                    po = ps.tile([V, TC], F32, tag="ps", name="ps")
                    for kc in range(NCH):
                        mm(po, wo[kc],
                           h[b][kc][:, 2 + ch * TC:2 + (ch + 1) * TC],
                           kc == 0, False)
                    for kd in range(NCH):
                        mm(po, wor[kd], em[kd], False, kd == NCH - 1)
                    ot = otp.tile([V, TC], F32, tag="otp", name="otp")
                    nc.scalar.activation(out=ot, in_=po, func=AF.Identity,
                                         bias=bout_t, scale=1.0)
                    nc.sync.dma_start(out=out[b, :, ch * TC:(ch + 1) * TC],
                                      in_=ot)

        es.close()

    nc.compile()
    return nc


def _block_lhsT(w):
    """(Cin, Cout) weight -> [kc, m, 128, 128] lhsT blocks."""
    ci, co = w.shape
    return np.ascontiguousarray(
        w.reshape(ci // P, P, co // P, P).transpose(0, 2, 1, 3))


def host_prep(inputs):
    """Build the 8 per-core input maps from the full problem inputs."""
    f = lambda x: np.asarray(x, dtype=np.float32)
    labels = np.asarray(inputs["labels"]).astype(np.int64)  # (T, B)
    enc_seq = f(inputs["enc_seq"])                          # (S, B, E)
    label_embed_W = f(inputs["label_embed_W"])
    time_embed_W = f(inputs["time_embed_W"])

    conv_glu_w = f(inputs["conv_glu_w"])    # (L, Cout, Cin, K)
    conv_id_w = f(inputs["conv_id_w"])
    res_proj_w = f(inputs["res_proj_w"])    # (L, Cout, Cin)
    inres_w = f(inputs["inres_w"])          # (L, C, D)
    in2enc_w = f(inputs["in2enc_w"])        # (L, E, C)
    lab2enc_w = f(inputs["lab2enc_w"])      # (L, E, D)
    enc2in_w = f(inputs["enc2in_w"])        # (L, C, E)
    out_res_w = f(inputs["out_res_w"])      # (V, D)
    out_proj_w = f(inputs["out_proj_w"])    # (V, C)

    Wglu = np.stack([[_block_lhsT(conv_glu_w[i, :, :, k].T) for k in range(KW)]
                     for i in range(L)])
    Wid = np.stack([[_block_lhsT(conv_id_w[i, :, :, k].T) for k in range(KW)]
                    for i in range(L)])
    Wres = np.stack([_block_lhsT(res_proj_w[i].T) for i in range(L)])
    Winres = np.stack([_block_lhsT(inres_w[i].T) for i in range(L)])
    Win2enc = np.stack([_block_lhsT(in2enc_w[i].T) for i in range(L)])
    Wlab2enc = np.stack([_block_lhsT(lab2enc_w[i].T) for i in range(L)])
    Wenc2in_r = np.ascontiguousarray(
        np.stack([enc2in_w[i].T for i in range(L)]).reshape(L, NCH, P, C))
    timeT = np.ascontiguousarray(time_embed_W.T.reshape(NCH, P, T))
    Wout = np.ascontiguousarray(out_proj_w.T.reshape(NCH, P, V))
    Woutres = np.ascontiguousarray(out_res_w.T.reshape(NCH, P, V))

    bglu = f(inputs["conv_glu_b"]).reshape(L, NCH, P, 1)
    bid = f(inputs["conv_id_b"]).reshape(L, NCH, P, 1)
    bres = f(inputs["res_proj_b"]).reshape(L, NCH, P, 1)
    bbeta = (f(inputs["in2enc_b"]) + f(inputs["lab2enc_b"])).reshape(L, NCH, P, 1)
    b6 = (f(inputs["inres_b"]) + f(inputs["enc2in_b"])).reshape(L, NCH, P, 1)
    bout = (f(inputs["out_proj_b"]) + f(inputs["out_res_b"])).reshape(V, 1)

    shared = dict(Wglu=Wglu, Wid=Wid, Wres=Wres, Winres=Winres,
                  Win2enc=Win2enc, Wlab2enc=Wlab2enc, Wenc2in_r=Wenc2in_r,
                  labelW=label_embed_W, timeT=timeT, Wout=Wout,
                  Woutres=Woutres, bglu=bglu, bid=bid, bres=bres,
                  bbeta=bbeta, b6=b6, bout=bout,
                  onesv=np.ones((P, 2), np.float32),
                  zerov=np.zeros((P, 2), np.float32))

    in_maps = []
    for c in range(NCORES):
        bsel = [c * BPC + p for p in range(BPC)]
        oh = np.zeros((BPC, V, T), np.float32)
        for p, bb in enumerate(bsel):
            oh[p, labels[:, bb], np.arange(T)] = 1.0
        encs = [np.ascontiguousarray(enc_seq[:, bb, :]) for bb in bsel]
        enc_lhs = np.stack([
            e.reshape(NCH, P, NCH, P).transpose(0, 2, 1, 3) for e in encs])
        enc_rhs = np.stack([e.reshape(NCH, P, E) for e in encs])
        m = dict(shared)
        m.update(onehot=oh, enc_lhs=enc_lhs, enc_rhs=enc_rhs)
        in_maps.append(m)
    return in_maps


def get_compiled():
    global _compiled
    if _compiled is None:
        _compiled = _build_nc()
    return _compiled


def kernel(**inputs):
    from concourse.bass_utils import run_bass_kernel_spmd

    nc = get_compiled()
    in_maps = host_prep(inputs)
    res = run_bass_kernel_spmd(nc, in_maps, list(range(NCORES)))
    out = np.empty((T, B, V), np.float32)
    for c in range(NCORES):
        o = res.results[c]["out"]  # (BPC, V, T)
        for p in range(BPC):
            out[:, c * BPC + p, :] = o[p].T
    return out
